# revision 11
# baseline (speedup 1.0000x reference)
"""Trainium2 Bass kernel for nn_AttentionNetwork (gnn_message_passing).

Computes, for f_meta [N, D] and W [2D, M] (N=4096, D=128, M=4):
    a = f_meta @ W[:D]            # [N, M]
    b = f_meta @ W[D:]            # [N, M]
    s = exp(relu(a[:,None,:] + b[None,:,:]))     # [N, N, M]
    out = s / sum(s, axis=-1, keepdims=True)

Key identity used on device:  exp(relu(x)) = max(exp(x), 1), and
exp(a+b) = exp(a)*exp(b).  So with ea = exp(a), eb = exp(b):
    t[i,j,m] = ea[i,m] * eb[j,m]
    s = max(t, 1) = relu(t - 1) + 1
    out = (r + 1) / (sum_m r + 4)     where r = relu(t - 1)

Sharding: row-parallel over source nodes i across 8 cores; each core
computes a [512, 4096, 4] slab. The N^2*M product grid is produced on
the TensorEngine as matmuls against an interleaved "selector" matrix
S[m, 4j+m'] = eb[j,m'] * delta(m,m') so the output is laid out
[i, (j,m)] with m innermost, making the 256MB of HBM writes fully
contiguous.
"""

import os
import sys

import numpy as np

for _p in ("/root/.axon_site/_ro/trn_rl_repo", "/opt/trn_rl_repo"):
    if os.path.isdir(_p) and _p not in sys.path:
        sys.path.append(_p)

import concourse.bass as bass
import concourse.mybir as mybir
import concourse.tile as tile
from concourse import bacc
from concourse.bass_utils import run_bass_kernel_spmd
from concourse.masks import make_identity

N = 4096          # number of nodes (j dimension)
D = 128           # feature dim
M = 4             # num meta paths
NCORES = 8
R = N // NCORES   # rows (i) per core = 512
F = N * M         # interleaved free size = 16384
FP = mybir.dt.float32

_CACHE = {}


def _build_nc():
    nc = bacc.Bacc(
        "TRN2",
        target_bir_lowering=False,
        debug=False,
        enable_asserts=False,
        num_devices=NCORES,
    )

    f_full = nc.dram_tensor("f_full", [N, D], FP, kind="ExternalInput").ap()
    f_mine = nc.dram_tensor("f_mine", [R, D], FP, kind="ExternalInput").ap()
    s_mask = nc.dram_tensor("s_mask", [M, F], FP, kind="ExternalInput").ap()
    out = nc.dram_tensor("out", [R, F], FP, kind="ExternalOutput").ap()

    w_dram = nc.dram_tensor("w", [2 * D, M], FP, kind="ExternalInput").ap()

    with tile.TileContext(nc) as tc:
        _emit(tc, out, f_full, f_mine, s_mask, w_dram)

    nc.compile()
    return nc


def _emit(tc, out, f_full, f_mine, s_mask, w_dram):
    nc = tc.nc
    AF = mybir.ActivationFunctionType
    OP = mybir.AluOpType

    n_fb = N // D            # 32 blocks of f_full
    n_mb = R // D            # 4 blocks of f_mine

    from contextlib import ExitStack
    ctx = ExitStack()
    # ---- persistent tiles -------------------------------------------------
    const_pool = ctx.enter_context(tc.tile_pool(name="const", bufs=1))
    bias_m1 = const_pool.tile([128, 1], FP)   # -1.0 for relu(t-1)
    nc.gpsimd.memset(bias_m1[:], -1.0)
    # ln on ACT only accepts |x| <= 2^64 but d+4 reaches ~4e36, so compute
    # g = ln((d+4) * 2^-64) and 1/(d+4) = exp(-g - 64*ln2).
    LNSCALE = 2.0 ** -64
    bias_4c = const_pool.tile([128, 1], FP)   # 4 * 2^-64 for ln((d+4)*c)
    nc.gpsimd.memset(bias_4c[:], 4.0 * LNSCALE)
    bias_mln = const_pool.tile([128, 1], FP)  # -64*ln2 for exp(-g - 64 ln2)
    nc.gpsimd.memset(bias_mln[:], -44.3614195558365)
    S = const_pool.tile([M, F], FP)        # interleaved selector matrix
    eaT = const_pool.tile([M, R], FP)      # exp(a_mine).T

    # ---- setup (transient tiles freed before steady state) ----------------
    with tc.tile_pool(name="setup_const", bufs=1) as scp, \
         tc.tile_pool(name="setup_ps", bufs=2, space="PSUM") as pst, \
         tc.tile_pool(name="setup_ps2", bufs=2, space="PSUM") as psb, \
         tc.tile_pool(name="setup_sb", bufs=3) as sbt:
        ident = scp.tile([128, 128], FP)
        make_identity(nc, ident[:])
        wa = scp.tile([D, M], FP)       # W[:D]
        wb = scp.tile([D, M], FP)       # W[D:]
        nc.sync.dma_start(wa[:], w_dram[0:D, :])
        nc.sync.dma_start(wb[:], w_dram[D:2 * D, :])

        # 0/1 interleave mask from host: mask[m, 4j+m'] = (m == m')
        mask = scp.tile([M, F], FP)
        nc.sync.dma_start(mask[:], s_mask[:, :])

        ebT = scp.tile([M, N], FP)      # exp(b).T
        # f tiles: [N, D] -> [128, n_fb*D] with block b at cols [b*D, (b+1)*D)
        ff = scp.tile([128, n_fb * D], FP)
        nc.sync.dma_start(
            ff[:].rearrange("p (b d) -> p b d", d=D),
            f_full.rearrange("(b p) d -> p b d", p=128),
        )
        fm = scp.tile([128, n_mb * D], FP)
        nc.sync.dma_start(
            fm[:].rearrange("p (b d) -> p b d", d=D),
            f_mine.rearrange("(b p) d -> p b d", p=128),
        )

        # ebT = exp(Wb.T @ f.T), eaT likewise
        for grp, (src, n_blk, w_t, dstT) in enumerate(
            ((ff, n_fb, wb, ebT), (fm, n_mb, wa, eaT))
        ):
            for r0 in range(0, n_blk, 8):
                nb = min(8, n_blk - r0)
                bp = psb.tile([M, 8 * D], FP, tag="bT")
                for q in range(nb):
                    k = r0 + q
                    tr = pst.tile([128, D], FP, tag="tr")
                    nc.tensor.transpose(tr[:], src[:, k * D:(k + 1) * D], ident[:])
                    fT = sbt.tile([128, D], FP, tag="fT")
                    nc.vector.tensor_copy(fT[:], tr[:])
                    nc.tensor.matmul(
                        bp[:, q * D:(q + 1) * D], w_t[:], fT[:],
                        start=True, stop=True,
                    )
                nc.scalar.activation(
                    dstT[:, r0 * D:(r0 + nb) * D], bp[:, 0:nb * D], AF.Exp,
                )

        # S = broadcast4(ebT) * mask  (chunked so it pipelines)
        SBCH = 2048
        for c0 in range(0, F, SBCH):
            j0 = c0 // M
            jn = SBCH // M
            nc.vector.tensor_tensor(
                S[:, c0:c0 + SBCH].rearrange("p (j m) -> p j m", m=M),
                ebT[:, j0:j0 + jn].broadcast_to((M, jn, M)),
                mask[:, c0:c0 + SBCH].rearrange("p (j m) -> p j m", m=M),
                op=OP.mult,
            )

    # ---- steady state -----------------------------------------------------
    # per i-block (128 rows), per half (8192 interleaved cols = 2048 j):
    #   PE:   t[128, 2048] = eaT_slice.T @ S_slice   (4 matmuls of 512)
    #   ACT:  r = relu(t - 1)                        (PSUM -> SBUF)
    #   GP:   rp = pairwise sum of r                 (stride-2 adds)
    #   DVE:  d  = pairwise sum of rp  -> sum_m r
    #   ACT:  g = ln(d + 4) ; eg = exp(-g)           (= 1/(d+4))
    #   DVE:  out = (r + 1) * eg[bcast4]             (in-place over r)
    #   DMA:  4MB contiguous store
    HALF = F // 2            # 8192
    JH = HALF // M           # 2048 j per half
    CH = 2048                # interleaved cols per chunk (one PSUM tensor)
    n_ch = HALF // CH        # 4 chunks per half

    with tc.tile_pool(name="ps_t", bufs=2, space="PSUM") as ps_t, \
         tc.tile_pool(name="big", bufs=2) as bigp, \
         tc.tile_pool(name="rp", bufs=4) as rpp, \
         tc.tile_pool(name="dsum", bufs=2) as dsp:
        for ib in range(R // 128):
            ea_sl = eaT[:, ib * 128:(ib + 1) * 128]
            for h in range(2):
                f0 = h * HALF
                big = bigp.tile([128, HALF], FP, tag="big")
                dsum = dsp.tile([128, JH], FP, tag="dsum")
                for c in range(n_ch):
                    tp = ps_t.tile([128, CH], FP, tag="tp")
                    for q in range(CH // 512):
                        co = c * CH + q * 512
                        nc.tensor.matmul(
                            tp[:, q * 512:(q + 1) * 512],
                            ea_sl, S[:, f0 + co:f0 + co + 512],
                            start=True, stop=True,
                        )
                    rr = big[:, c * CH:(c + 1) * CH]
                    # r = relu(t - 1)
                    nc.scalar.activation(rr, tp[:], AF.Relu, bias=bias_m1[:])
                    # pairwise adds: rp[k] = r[2k] + r[2k+1]
                    rp = rpp.tile([128, CH // 2], FP, tag="rp")
                    nc.gpsimd.tensor_tensor(
                        rp[:], rr[:, 0::2], rr[:, 1::2], op=OP.add,
                    )
                    nc.vector.tensor_tensor(
                        dsum[:, c * (CH // 4):(c + 1) * (CH // 4)],
                        rp[:, 0::2], rp[:, 1::2], op=OP.add,
                    )
                # 1/(d+4) via exp(-ln((d+4)c) - ln(1/c)) on ACT (keeps DVE free)
                nc.scalar.activation(dsum[:], dsum[:], AF.Ln,
                                     bias=bias_4c[:], scale=LNSCALE)
                nc.scalar.activation(dsum[:], dsum[:], AF.Exp,
                                     bias=bias_mln[:], scale=-1.0)
                # out = (r + 1) * recip, in place over big
                big3 = big[:].rearrange("p (j m) -> p j m", m=M)
                nc.vector.scalar_tensor_tensor(
                    big3, big3, 1.0, dsum[:].broadcast_to((128, JH, M)),
                    op0=OP.add, op1=OP.mult,
                )
                nc.sync.dma_start(
                    out[ib * 128:(ib + 1) * 128, f0:f0 + HALF], big[:],
                )
    ctx.close()


def kernel(f_meta: np.ndarray, W: np.ndarray) -> np.ndarray:
    f_meta = np.ascontiguousarray(f_meta, dtype=np.float32)
    W = np.ascontiguousarray(W, dtype=np.float32)
    assert f_meta.shape == (N, D) and W.shape == (2 * D, M)

    if "nc" not in _CACHE:
        _CACHE["nc"] = _build_nc()
    nc = _CACHE["nc"]

    mask = np.zeros((M, F), dtype=np.float32)
    for m in range(M):
        mask[m, m::M] = 1.0
    in_maps = [
        {
            "f_full": f_meta,
            "f_mine": np.ascontiguousarray(f_meta[c * R:(c + 1) * R]),
            "s_mask": mask,
            "w": W,
        }
        for c in range(NCORES)
    ]
    res = run_bass_kernel_spmd(nc, in_maps, core_ids=list(range(NCORES)))
    slabs = [res.results[c]["out"] for c in range(NCORES)]
    return np.concatenate(slabs, axis=0).reshape(N, N, M)


if __name__ == "__main__":
    f = np.random.randn(N, D).astype(np.float32)
    w = np.random.randn(2 * D, M).astype(np.float32)
    o = kernel(f, w)
    print(o.shape, o.dtype, o[0, 0], o.sum(axis=-1).mean())
